# revision 37
# baseline (speedup 1.0000x reference)
"""Boundary rendering module for Trainium2 (8 NeuronCores), fused single launch.

Computes, for x of shape (2, 4, 64, 256, 256) f32:
    mn/mx  = per-channel global min/max
    binary = ((x - mn) / (mx - mn)) > 0.5     [== (x - mn) > 0.5*(mx - mn)]
    dilated = 3x3x3 binary dilation of binary (SAME padding)
    out    = dilated - binary

Sharding: H (=256) split into 8 chunks of 32 rows, one per NeuronCore.
Each core receives its 32 rows plus one halo row on each side (global
edges padded with -1e30 so the halo mask is 0).  On-core layout puts
(B, D) = 128 on the SBUF partition axis; (C, H, W) live on the free axis.

Single launch per core:
  1. x loaded in 8 (channel, half) chunks via gpsimd SWDGE DMA (spreads
     across all 16 SDMA engines; the 2 HWDGE rings top out at ~75 GB/s).
  2. DVE min/max partials per chunk as each load lands (overlapped).
  3. Partials transposed across partitions with a PE identity matmul,
     reduced, then an 8-core AllReduce(max) over [mx(4) | -mn(4)]
     through DRAM bounce buffers; result broadcast back to all 128
     partitions with a rank-1 PE matmul.
  4. Mask + H-dilation on DVE (bf16), W+D dilation as banded PE matmuls
     accumulating a neighbor count in PSUM, minus 16*binary, then a
     saturated sigmoid on ScalarE emits exact {0.0, 1.0}.
  5. Stores via gpsimd SWDGE in 1 MiB chunks.
Compute runs at half-channel granularity (16 own rows) with
double-buffered mask/dilation tiles so DVE/PE/ScalarE/DMA pipeline.
"""

import os
import sys

import numpy as np

for _p in ("/opt/trn_rl_repo", "/root/.axon_site/_ro/trn_rl_repo"):
    if os.path.isdir(_p) and _p not in sys.path:
        sys.path.insert(0, _p)

import ml_dtypes

B, C, D, H, W = 2, 4, 64, 256, 256
NCORES = 8
HS = H // NCORES  # 32 own rows per core
HA = HS + 2  # rows incl halo
HPAD = np.float32(-1e30)  # halo pad at global H edges -> mask 0

MHW = 258  # mH row width: 256 data cols + 2 zero pad cols
# half-channel mH: 17 rows (1 pad + 16 data) + slack for dw=+1 AP views
MHLEN = 17 * MHW + 2

# flat per-partition sizes: DRAM shards use layout [B, D, C, H', W] so each
# partition (b, d) owns one contiguous run -> 2-dim DMA APs.  Strided 3-dim
# APs run ~4x slower through SWDGE (measured 244us vs 56us for the 17.8 MiB
# load) and defeat chunk-completion staggering.
XPP = C * HA * W  # 34816 f32 per partition (input incl halo)
OPP = C * HS * W  # 32768 f32 per partition (output)

_TWO_PHASE = False
_CACHE = {}


def _consts():
    bd = np.arange(128)
    b = bd // D
    d = bd % D
    A = (b[:, None] == b[None, :]) & (np.abs(d[:, None] - d[None, :]) <= 1)
    A = A.astype(ml_dtypes.bfloat16)
    negI = (-16.0 * np.eye(128)).astype(ml_dtypes.bfloat16)
    I128 = np.eye(128, dtype=np.float32)
    return A, negI, I128


def _build():
    import concourse.bass as bass
    import concourse.bacc as bacc
    import concourse.bass_isa as bass_isa
    import concourse.mybir as mybir
    import concourse.tile as tile
    from contextlib import ExitStack

    f32 = mybir.dt.float32
    bf16 = mybir.dt.bfloat16
    Alu = mybir.AluOpType

    nc = bacc.Bacc(
        "TRN2",
        target_bir_lowering=False,
        debug=False,
        num_devices=NCORES,
    )

    xs = nc.dram_tensor("xs", [B, D, XPP], f32, kind="ExternalInput")
    out = nc.dram_tensor("out", [B, D, OPP], f32, kind="ExternalOutput")
    A_np, negI_np, I_np = _consts()
    bandA_d = nc.inline_tensor(A_np, name="bandA")
    negI_d = nc.inline_tensor(negI_np, name="negI")
    ident_d = nc.inline_tensor(I_np, name="ident")

    # partition axis = (b, d) = 128; the (b, d) DRAM dims merge into a single
    # 128-partition dim during AP optimization since b's stride = 64 * d's.
    xsa = xs.ap()
    outa = out.ap()

    with ExitStack() as ctx:
        tc = ctx.enter_context(tile.TileContext(nc))
        pers = ctx.enter_context(tc.tile_pool(name="pers", bufs=1))
        maskp = ctx.enter_context(tc.tile_pool(name="mask", bufs=2))
        stagp = ctx.enter_context(tc.tile_pool(name="stag", bufs=2))
        psump = ctx.enter_context(tc.tile_pool(name="psum", bufs=4, space="PSUM"))
        dramp = ctx.enter_context(tc.tile_pool(name="dram", bufs=1, space="DRAM"))

        x_all = pers.tile([128, C, HA, W], f32)  # 136 KiB / partition
        pmax = pers.tile([128, 16], f32)
        pmin = pers.tile([128, 16], f32)
        red8 = pers.tile([128, 8], f32)  # per channel c: [mx_c, -mn_c] at 2c
        par8 = pers.tile([128, 8], f32)  # cross-partition reduced partials
        s1v = pers.tile([128, 8], f32)  # allreduced vals on partition 0
        gv8 = pers.tile([128, 8], f32)  # broadcast [mx, -mn] x 4 channels
        mnv = pers.tile([128, 4], f32)  # mn per channel
        h4 = pers.tile([128, 4], f32)  # 0.5*(mx-mn) per channel
        At = pers.tile([128, 128], bf16)
        Nt = pers.tile([128, 128], bf16)
        sel_bias = pers.tile([128, 2], f32)  # col 0: sigmoid bias; also dummy-AR payload

        nc.vector.memset(sel_bias[:, :], -100.0)
        nc.sync.dma_start(out=At[:, :], in_=bandA_d.ap())
        nc.sync.dma_start(out=Nt[:, :], in_=negI_d.ap())

        # ---- bulk loads: 8 chunks of 2.2 MiB on the gpsimd SWDGE queue ----
        # FIFO ring order staggers completions so per-chunk reduces overlap.
        xv = x_all[:].rearrange("p c h w -> p (c h w)")
        # channel 0 loads in 4 half-chunks so the first DVE reduce starts
        # ~4us earlier; later channels use full 17-row chunks
        for c in range(C):
            nsub = 2 if c == 0 else 1
            for hf in range(2):
                o = c * (HA * W) + hf * 17 * W
                step = (17 * W) // nsub
                for u in range(nsub):
                    nc.gpsimd.dma_start(
                        out=xv[:, o + u * step : o + (u + 1) * step],
                        in_=xsa[:, :, o + u * step : o + (u + 1) * step],
                    )
        # zero the pad columns of both mH pool buffers once, on gpsimd while
        # it idles behind the load descriptor generation (off DVE's path)
        for _ in range(2):
            mh0 = maskp.tile([128, MHLEN], bf16, tag="mh")
            nc.gpsimd.memset(mh0[:, :], 0.0)

        # dummy AllReduce fired as early as possible: absorbs the collective
        # startup + cross-core launch skew so the real ARs' meshes run at
        # their pipelined ~10-20us cost instead of waiting on the ~76us floor
        dmy_in = dramp.tile([1, 2], f32, tag="dmyin")
        dmy_out = dramp.tile([1, 2], f32, tag="dmyout")
        nc.sync.dma_start(out=dmy_in[:, :], in_=sel_bias[0:1, 0:2])
        nc.gpsimd.collective_compute(
            "AllReduce",
            Alu.max,
            replica_groups=[list(range(NCORES))],
            ins=[dmy_in[:, :].opt()],
            outs=[dmy_out[:, :].opt()],
        )

        # ---- phase 1: min/max partials + 3 pipelined AllReduces ----
        # Channel 0 gets its own AllReduce so the first mesh collective
        # starts as early as possible (its ~27us latency hides under the
        # remaining min/max passes); later groups' meshes pipeline ~10us
        # apart behind it.
        groups = [(0,), (1,), (2, 3)]
        # own-row subranges per channel; channel 0 is split finer so the
        # first reduces start as soon as its first half-chunk lands
        subranges = {0: [(1, 8), (8, 17), (17, 25), (25, 33)]}
        for g, chans in enumerate(groups):
            for c in chans:
                subs = subranges.get(c, [(1, 17), (17, 33)])
                for k, (ra, rb) in enumerate(subs):
                    o1 = c * (HA * W) + ra * W
                    rows = xv[:, o1 : o1 + (rb - ra) * W]
                    nc.vector.tensor_reduce(
                        out=pmax[:, 4 * c + k : 4 * c + k + 1],
                        in_=rows,
                        axis=mybir.AxisListType.X,
                        op=Alu.max,
                    )
                    nc.vector.tensor_reduce(
                        out=pmin[:, 4 * c + k : 4 * c + k + 1],
                        in_=rows,
                        axis=mybir.AxisListType.X,
                        op=Alu.min,
                    )
                nc.vector.tensor_reduce(
                    out=red8[:, 2 * c : 2 * c + 1],
                    in_=pmax[:, 4 * c : 4 * c + len(subs)],
                    axis=mybir.AxisListType.X,
                    op=Alu.max,
                )
                nc.vector.tensor_reduce(
                    out=red8[:, 2 * c + 1 : 2 * c + 2],
                    in_=pmin[:, 4 * c : 4 * c + len(subs)],
                    axis=mybir.AxisListType.X,
                    op=Alu.min,
                )
                # negate the min so a single AllReduce(max) covers both
                nc.vector.tensor_scalar_mul(
                    red8[:, 2 * c + 1 : 2 * c + 2],
                    red8[:, 2 * c + 1 : 2 * c + 2],
                    -1.0,
                )
            c0 = chans[0]
            wid = 2 * len(chans)
            # cross-partition max of the group's partials on gpsimd, then
            # DMA out partition 0 and AllReduce(max) across the 8 cores.
            nc.gpsimd.partition_all_reduce(
                out_ap=par8[:, 2 * c0 : 2 * c0 + wid],
                in_ap=red8[:, 2 * c0 : 2 * c0 + wid],
                channels=128,
                reduce_op=bass_isa.ReduceOp.max,
            )
            cc_in = dramp.tile([1, wid], f32, tag=f"ccin{g}")
            cc_out = dramp.tile([1, wid], f32, tag=f"ccout{g}")
            nc.scalar.dma_start(
                out=cc_in[:, :], in_=par8[0:1, 2 * c0 : 2 * c0 + wid]
            )
            nc.gpsimd.collective_compute(
                "AllReduce",
                Alu.max,
                replica_groups=[list(range(NCORES))],
                ins=[cc_in[:, :].opt()],
                outs=[cc_out[:, :].opt()],
            )
            nc.sync.dma_start(
                out=s1v[0:1, 2 * c0 : 2 * c0 + wid],
                in_=cc_out[:, :],
            )

        # ---- phase 2: per group: thresholds, mask, dilate, boundary ----
        # all broadcasts emitted first: gpsimd's in-order program must not
        # park a later group's broadcast behind an earlier group's stores
        for g, chans in enumerate(groups):
            c0 = chans[0]
            wid = 2 * len(chans)
            nc.gpsimd.partition_broadcast(
                out_ap=gv8[:, 2 * c0 : 2 * c0 + wid],
                in_ap=s1v[0:1, 2 * c0 : 2 * c0 + wid],
            )
        for g, chans in enumerate(groups):
            for c in chans:
                nc.vector.tensor_scalar_mul(
                    mnv[:, c : c + 1],
                    gv8[:, 2 * c + 1 : 2 * c + 2],
                    -1.0,
                )
                nc.vector.tensor_add(
                    h4[:, c : c + 1],
                    gv8[:, 2 * c : 2 * c + 1],
                    gv8[:, 2 * c + 1 : 2 * c + 2],
                )
                nc.vector.tensor_scalar_mul(h4[:, c : c + 1], h4[:, c : c + 1], 0.5)
            for c in chans:
                for hf in range(2):
                    # binm rows 0..17 = x halo rows 16*hf .. 16*hf+17
                    binm = maskp.tile([128, 18, W], bf16, tag="bin")
                    mH = maskp.tile([128, MHLEN], bf16, tag="mh")
                    nc.vector.tensor_scalar(
                        out=binm[:, :, :],
                        in0=x_all[:, c, 16 * hf : 16 * hf + 18, :],
                        scalar1=mnv[:, c : c + 1],
                        scalar2=h4[:, c : c + 1],
                        op0=Alu.subtract,
                        op1=Alu.is_gt,
                    )
                    mHd = mH[:, MHW : MHW + 16 * MHW].rearrange(
                        "p (r z) -> p r z", z=MHW
                    )[:, :, 0:W]
                    # H-max in two row-ranges (0:9 covers PE groups tp=0's
                    # mH reads incl the row-9 overlap) so the first PE
                    # groups start before the second range finishes
                    for ra, rb in ((0, 9), (9, 16)):
                        nc.vector.tensor_tensor(
                            out=mHd[:, ra:rb, :],
                            in0=binm[:, ra : rb, :],
                            in1=binm[:, ra + 2 : rb + 2, :],
                            op=Alu.max,
                        )
                        nc.vector.tensor_tensor(
                            out=mHd[:, ra:rb, :],
                            in0=mHd[:, ra:rb, :],
                            in1=binm[:, ra + 1 : rb + 1, :],
                            op=Alu.max,
                        )
                    for tp in range(2):  # 8 own rows per staging buffer
                        stg = stagp.tile([128, 2048], f32, tag="st")
                        for t in range(2):  # 4 own rows per PSUM tile
                            ps = psump.tile([128, 1024], f32, tag="ps")
                            for s in range(2):  # one PSUM bank = 2 rows
                                R = 8 * tp + 4 * t + 2 * s
                                pslice = ps[:, 512 * s : 512 * s + 512]
                                for jj, dw in enumerate((-1, 0, 1)):
                                    off = (R + 1) * MHW + dw
                                    rhs = mH[:, off : off + 2 * MHW].rearrange(
                                        "p (r z) -> p r z", z=MHW
                                    )[:, :, 0:W]
                                    nc.tensor.matmul(
                                        pslice,
                                        At[:, :],
                                        rhs,
                                        start=(jj == 0),
                                        stop=False,
                                    )
                                nc.tensor.matmul(
                                    pslice,
                                    Nt[:, :],
                                    binm[:, 1 + R : 3 + R, :],
                                    start=False,
                                    stop=True,
                                )
                            nc.scalar.activation(
                                out=stg[:, 1024 * t : 1024 * t + 1024],
                                in_=ps[:, :],
                                func=mybir.ActivationFunctionType.Sigmoid,
                                bias=sel_bias[:, 0:1],
                                scale=200.0,
                            )
                        r0 = 16 * hf + 8 * tp  # own-row base in the shard
                        oo = c * (HS * W) + r0 * W
                        nc.gpsimd.dma_start(
                            out=outa[:, :, oo : oo + 8 * W],
                            in_=stg[:, :],
                        )

    nc.compile()
    return nc


def _get_nc():
    if "nc" not in _CACHE:
        _CACHE["nc"] = _build()
    return _CACHE["nc"]


def _make_in_maps(x: np.ndarray):
    # device shards use layout [B, D, C, HA, W] (flat per partition)
    xt = np.ascontiguousarray(x.transpose(0, 2, 1, 3, 4))  # [B, D, C, H, W]
    in_maps = []
    for k in range(NCORES):
        xs = np.empty((B, D, C, HA, W), np.float32)
        lo = k * HS
        xs[:, :, :, 1 : HS + 1, :] = xt[:, :, :, lo : lo + HS, :]
        if k > 0:
            xs[:, :, :, 0, :] = xt[:, :, :, lo - 1, :]
        else:
            xs[:, :, :, 0, :] = HPAD
        if k < NCORES - 1:
            xs[:, :, :, HS + 1, :] = xt[:, :, :, lo + HS, :]
        else:
            xs[:, :, :, HS + 1, :] = HPAD
        in_maps.append({"xs": xs.reshape(B, D, XPP)})
    return in_maps


def kernel(x: np.ndarray) -> np.ndarray:
    from concourse.bass_utils import run_bass_kernel_spmd

    x = np.ascontiguousarray(np.asarray(x), dtype=np.float32)
    assert x.shape == (B, C, D, H, W)

    in_maps = _make_in_maps(x)
    res = run_bass_kernel_spmd(_get_nc(), in_maps, core_ids=list(range(NCORES)))
    # shard outs are [B, D, C, HS, W]; back to [B, C, D, HS, W], concat on H
    pieces = [
        res.results[k]["out"].reshape(B, D, C, HS, W).transpose(0, 2, 1, 3, 4)
        for k in range(NCORES)
    ]
    return np.concatenate(pieces, axis=3)


if __name__ == "__main__":
    x = np.random.randn(B, C, D, H, W).astype(np.float32)
    y = kernel(x)
    print(y.shape, y.dtype, y.sum())


# revision 38
# speedup vs baseline: 1.0575x; 1.0575x over previous
"""Boundary rendering module for Trainium2 (8 NeuronCores), fused single launch.

Computes, for x of shape (2, 4, 64, 256, 256) f32:
    mn/mx  = per-channel global min/max
    binary = ((x - mn) / (mx - mn)) > 0.5     [== (x - mn) > 0.5*(mx - mn)]
    dilated = 3x3x3 binary dilation of binary (SAME padding)
    out    = dilated - binary

Sharding: H (=256) split into 8 chunks of 32 rows, one per NeuronCore.
Each core receives its 32 rows plus one halo row on each side (global
edges padded with -1e30 so the halo mask is 0).  On-core layout puts
(B, D) = 128 on the SBUF partition axis; (C, H, W) live on the free axis.

Single launch per core:
  1. x loaded in 8 (channel, half) chunks via gpsimd SWDGE DMA (spreads
     across all 16 SDMA engines; the 2 HWDGE rings top out at ~75 GB/s).
  2. DVE min/max partials per chunk as each load lands (overlapped).
  3. Partials transposed across partitions with a PE identity matmul,
     reduced, then an 8-core AllReduce(max) over [mx(4) | -mn(4)]
     through DRAM bounce buffers; result broadcast back to all 128
     partitions with a rank-1 PE matmul.
  4. Mask + H-dilation on DVE (bf16), W+D dilation as banded PE matmuls
     accumulating a neighbor count in PSUM, minus 16*binary, then a
     saturated sigmoid on ScalarE emits exact {0.0, 1.0}.
  5. Stores via gpsimd SWDGE in 1 MiB chunks.
Compute runs at half-channel granularity (16 own rows) with
double-buffered mask/dilation tiles so DVE/PE/ScalarE/DMA pipeline.
"""

import os
import sys

import numpy as np

for _p in ("/opt/trn_rl_repo", "/root/.axon_site/_ro/trn_rl_repo"):
    if os.path.isdir(_p) and _p not in sys.path:
        sys.path.insert(0, _p)

import ml_dtypes

B, C, D, H, W = 2, 4, 64, 256, 256
NCORES = 8
HS = H // NCORES  # 32 own rows per core
HA = HS + 2  # rows incl halo
HPAD = np.float32(-1e30)  # halo pad at global H edges -> mask 0

MHW = 258  # mH row width: 256 data cols + 2 zero pad cols
# half-channel mH: 17 rows (1 pad + 16 data) + slack for dw=+1 AP views
MHLEN = 17 * MHW + 2

# flat per-partition sizes: DRAM shards use layout [B, D, C, H', W] so each
# partition (b, d) owns one contiguous run -> 2-dim DMA APs.  Strided 3-dim
# APs run ~4x slower through SWDGE (measured 244us vs 56us for the 17.8 MiB
# load) and defeat chunk-completion staggering.
XPP = C * HA * W  # 34816 f32 per partition (input incl halo)
OPP = C * HS * W  # 32768 f32 per partition (output)

_TWO_PHASE = False
_CACHE = {}


def _consts():
    bd = np.arange(128)
    b = bd // D
    d = bd % D
    A = (b[:, None] == b[None, :]) & (np.abs(d[:, None] - d[None, :]) <= 1)
    A = A.astype(ml_dtypes.bfloat16)
    negI = (-16.0 * np.eye(128)).astype(ml_dtypes.bfloat16)
    I128 = np.eye(128, dtype=np.float32)
    return A, negI, I128


def _build():
    import concourse.bass as bass
    import concourse.bacc as bacc
    import concourse.bass_isa as bass_isa
    import concourse.mybir as mybir
    import concourse.tile as tile
    from contextlib import ExitStack

    f32 = mybir.dt.float32
    bf16 = mybir.dt.bfloat16
    Alu = mybir.AluOpType

    nc = bacc.Bacc(
        "TRN2",
        target_bir_lowering=False,
        debug=False,
        num_devices=NCORES,
    )

    xs = nc.dram_tensor("xs", [B, D, XPP], f32, kind="ExternalInput")
    out = nc.dram_tensor("out", [B, D, OPP], f32, kind="ExternalOutput")
    A_np, negI_np, I_np = _consts()
    bandA_d = nc.inline_tensor(A_np, name="bandA")
    negI_d = nc.inline_tensor(negI_np, name="negI")
    ident_d = nc.inline_tensor(I_np, name="ident")

    # partition axis = (b, d) = 128; the (b, d) DRAM dims merge into a single
    # 128-partition dim during AP optimization since b's stride = 64 * d's.
    xsa = xs.ap()
    outa = out.ap()

    with ExitStack() as ctx:
        tc = ctx.enter_context(tile.TileContext(nc))
        pers = ctx.enter_context(tc.tile_pool(name="pers", bufs=1))
        maskp = ctx.enter_context(tc.tile_pool(name="mask", bufs=2))
        stagp = ctx.enter_context(tc.tile_pool(name="stag", bufs=2))
        psump = ctx.enter_context(tc.tile_pool(name="psum", bufs=4, space="PSUM"))
        dramp = ctx.enter_context(tc.tile_pool(name="dram", bufs=1, space="DRAM"))

        x_all = pers.tile([128, C, HA, W], f32)  # 136 KiB / partition
        pmax = pers.tile([128, 16], f32)
        pmin = pers.tile([128, 16], f32)
        red8 = pers.tile([128, 8], f32)  # per channel c: [mx_c, -mn_c] at 2c
        par8 = pers.tile([128, 8], f32)  # cross-partition reduced partials
        s1v = pers.tile([128, 8], f32)  # allreduced vals on partition 0
        gv8 = pers.tile([128, 8], f32)  # broadcast [mx, -mn] x 4 channels
        mnv = pers.tile([128, 4], f32)  # mn per channel
        h4 = pers.tile([128, 4], f32)  # 0.5*(mx-mn) per channel
        At = pers.tile([128, 128], bf16)
        Nt = pers.tile([128, 128], bf16)
        sel_bias = pers.tile([128, 2], f32)  # col 0: sigmoid bias; also dummy-AR payload

        nc.vector.memset(sel_bias[:, :], -100.0)
        nc.sync.dma_start(out=At[:, :], in_=bandA_d.ap())
        nc.sync.dma_start(out=Nt[:, :], in_=negI_d.ap())

        # ---- bulk loads: 8 chunks of 2.2 MiB on the gpsimd SWDGE queue ----
        # FIFO ring order staggers completions so per-chunk reduces overlap.
        xv = x_all[:].rearrange("p c h w -> p (c h w)")
        # channel 0 loads in 4 half-chunks so the first DVE reduce starts
        # ~4us earlier; later channels use full 17-row chunks
        for c in range(C):
            nsub = 2 if c == 0 else 1
            for hf in range(2):
                o = c * (HA * W) + hf * 17 * W
                step = (17 * W) // nsub
                for u in range(nsub):
                    nc.gpsimd.dma_start(
                        out=xv[:, o + u * step : o + (u + 1) * step],
                        in_=xsa[:, :, o + u * step : o + (u + 1) * step],
                    )
        # zero the pad columns of both mH pool buffers once, on gpsimd while
        # it idles behind the load descriptor generation (off DVE's path)
        for _ in range(2):
            mh0 = maskp.tile([128, MHLEN], bf16, tag="mh")
            nc.gpsimd.memset(mh0[:, :], 0.0)

        # ---- phase 1: min/max partials + 3 pipelined AllReduces ----
        # Channel 0 gets its own AllReduce so the first mesh collective
        # starts as early as possible (its ~27us latency hides under the
        # remaining min/max passes); later groups' meshes pipeline ~10us
        # apart behind it.
        groups = [(0,), (1,), (2, 3)]
        # own-row subranges per channel; channel 0 is split finer so the
        # first reduces start as soon as its first half-chunk lands
        subranges = {0: [(1, 8), (8, 17), (17, 25), (25, 33)]}
        for g, chans in enumerate(groups):
            for c in chans:
                subs = subranges.get(c, [(1, 17), (17, 33)])
                for k, (ra, rb) in enumerate(subs):
                    o1 = c * (HA * W) + ra * W
                    rows = xv[:, o1 : o1 + (rb - ra) * W]
                    nc.vector.tensor_reduce(
                        out=pmax[:, 4 * c + k : 4 * c + k + 1],
                        in_=rows,
                        axis=mybir.AxisListType.X,
                        op=Alu.max,
                    )
                    nc.vector.tensor_reduce(
                        out=pmin[:, 4 * c + k : 4 * c + k + 1],
                        in_=rows,
                        axis=mybir.AxisListType.X,
                        op=Alu.min,
                    )
                nc.vector.tensor_reduce(
                    out=red8[:, 2 * c : 2 * c + 1],
                    in_=pmax[:, 4 * c : 4 * c + len(subs)],
                    axis=mybir.AxisListType.X,
                    op=Alu.max,
                )
                nc.vector.tensor_reduce(
                    out=red8[:, 2 * c + 1 : 2 * c + 2],
                    in_=pmin[:, 4 * c : 4 * c + len(subs)],
                    axis=mybir.AxisListType.X,
                    op=Alu.min,
                )
                # negate the min so a single AllReduce(max) covers both
                nc.vector.tensor_scalar_mul(
                    red8[:, 2 * c + 1 : 2 * c + 2],
                    red8[:, 2 * c + 1 : 2 * c + 2],
                    -1.0,
                )
            c0 = chans[0]
            wid = 2 * len(chans)
            # cross-partition max of the group's partials on gpsimd, then
            # DMA out partition 0 and AllReduce(max) across the 8 cores.
            nc.gpsimd.partition_all_reduce(
                out_ap=par8[:, 2 * c0 : 2 * c0 + wid],
                in_ap=red8[:, 2 * c0 : 2 * c0 + wid],
                channels=128,
                reduce_op=bass_isa.ReduceOp.max,
            )
            cc_in = dramp.tile([1, wid], f32, tag=f"ccin{g}")
            cc_out = dramp.tile([1, wid], f32, tag=f"ccout{g}")
            nc.scalar.dma_start(
                out=cc_in[:, :], in_=par8[0:1, 2 * c0 : 2 * c0 + wid]
            )
            nc.gpsimd.collective_compute(
                "AllReduce",
                Alu.max,
                replica_groups=[list(range(NCORES))],
                ins=[cc_in[:, :].opt()],
                outs=[cc_out[:, :].opt()],
            )
            nc.sync.dma_start(
                out=s1v[0:1, 2 * c0 : 2 * c0 + wid],
                in_=cc_out[:, :],
            )

        # ---- phase 2: per group: thresholds, mask, dilate, boundary ----
        # all broadcasts emitted first: gpsimd's in-order program must not
        # park a later group's broadcast behind an earlier group's stores
        for g, chans in enumerate(groups):
            c0 = chans[0]
            wid = 2 * len(chans)
            nc.gpsimd.partition_broadcast(
                out_ap=gv8[:, 2 * c0 : 2 * c0 + wid],
                in_ap=s1v[0:1, 2 * c0 : 2 * c0 + wid],
            )
        for g, chans in enumerate(groups):
            for c in chans:
                nc.vector.tensor_scalar_mul(
                    mnv[:, c : c + 1],
                    gv8[:, 2 * c + 1 : 2 * c + 2],
                    -1.0,
                )
                nc.vector.tensor_add(
                    h4[:, c : c + 1],
                    gv8[:, 2 * c : 2 * c + 1],
                    gv8[:, 2 * c + 1 : 2 * c + 2],
                )
                nc.vector.tensor_scalar_mul(h4[:, c : c + 1], h4[:, c : c + 1], 0.5)
            for c in chans:
                for hf in range(2):
                    # binm rows 0..17 = x halo rows 16*hf .. 16*hf+17
                    binm = maskp.tile([128, 18, W], bf16, tag="bin")
                    mH = maskp.tile([128, MHLEN], bf16, tag="mh")
                    nc.vector.tensor_scalar(
                        out=binm[:, :, :],
                        in0=x_all[:, c, 16 * hf : 16 * hf + 18, :],
                        scalar1=mnv[:, c : c + 1],
                        scalar2=h4[:, c : c + 1],
                        op0=Alu.subtract,
                        op1=Alu.is_gt,
                    )
                    mHd = mH[:, MHW : MHW + 16 * MHW].rearrange(
                        "p (r z) -> p r z", z=MHW
                    )[:, :, 0:W]
                    # H-max in two row-ranges (0:9 covers PE groups tp=0's
                    # mH reads incl the row-9 overlap) so the first PE
                    # groups start before the second range finishes
                    for ra, rb in ((0, 9), (9, 16)):
                        nc.vector.tensor_tensor(
                            out=mHd[:, ra:rb, :],
                            in0=binm[:, ra : rb, :],
                            in1=binm[:, ra + 2 : rb + 2, :],
                            op=Alu.max,
                        )
                        nc.vector.tensor_tensor(
                            out=mHd[:, ra:rb, :],
                            in0=mHd[:, ra:rb, :],
                            in1=binm[:, ra + 1 : rb + 1, :],
                            op=Alu.max,
                        )
                    for tp in range(2):  # 8 own rows per staging buffer
                        stg = stagp.tile([128, 2048], f32, tag="st")
                        for t in range(2):  # 4 own rows per PSUM tile
                            ps = psump.tile([128, 1024], f32, tag="ps")
                            for s in range(2):  # one PSUM bank = 2 rows
                                R = 8 * tp + 4 * t + 2 * s
                                pslice = ps[:, 512 * s : 512 * s + 512]
                                for jj, dw in enumerate((-1, 0, 1)):
                                    off = (R + 1) * MHW + dw
                                    rhs = mH[:, off : off + 2 * MHW].rearrange(
                                        "p (r z) -> p r z", z=MHW
                                    )[:, :, 0:W]
                                    nc.tensor.matmul(
                                        pslice,
                                        At[:, :],
                                        rhs,
                                        start=(jj == 0),
                                        stop=False,
                                    )
                                nc.tensor.matmul(
                                    pslice,
                                    Nt[:, :],
                                    binm[:, 1 + R : 3 + R, :],
                                    start=False,
                                    stop=True,
                                )
                            nc.scalar.activation(
                                out=stg[:, 1024 * t : 1024 * t + 1024],
                                in_=ps[:, :],
                                func=mybir.ActivationFunctionType.Sigmoid,
                                bias=sel_bias[:, 0:1],
                                scale=200.0,
                            )
                        r0 = 16 * hf + 8 * tp  # own-row base in the shard
                        oo = c * (HS * W) + r0 * W
                        nc.gpsimd.dma_start(
                            out=outa[:, :, oo : oo + 8 * W],
                            in_=stg[:, :],
                        )

    nc.compile()
    return nc


def _get_nc():
    if "nc" not in _CACHE:
        _CACHE["nc"] = _build()
    return _CACHE["nc"]


def _make_in_maps(x: np.ndarray):
    # device shards use layout [B, D, C, HA, W] (flat per partition)
    xt = np.ascontiguousarray(x.transpose(0, 2, 1, 3, 4))  # [B, D, C, H, W]
    in_maps = []
    for k in range(NCORES):
        xs = np.empty((B, D, C, HA, W), np.float32)
        lo = k * HS
        xs[:, :, :, 1 : HS + 1, :] = xt[:, :, :, lo : lo + HS, :]
        if k > 0:
            xs[:, :, :, 0, :] = xt[:, :, :, lo - 1, :]
        else:
            xs[:, :, :, 0, :] = HPAD
        if k < NCORES - 1:
            xs[:, :, :, HS + 1, :] = xt[:, :, :, lo + HS, :]
        else:
            xs[:, :, :, HS + 1, :] = HPAD
        in_maps.append({"xs": xs.reshape(B, D, XPP)})
    return in_maps


def kernel(x: np.ndarray) -> np.ndarray:
    from concourse.bass_utils import run_bass_kernel_spmd

    x = np.ascontiguousarray(np.asarray(x), dtype=np.float32)
    assert x.shape == (B, C, D, H, W)

    in_maps = _make_in_maps(x)
    res = run_bass_kernel_spmd(_get_nc(), in_maps, core_ids=list(range(NCORES)))
    # shard outs are [B, D, C, HS, W]; back to [B, C, D, HS, W], concat on H
    pieces = [
        res.results[k]["out"].reshape(B, D, C, HS, W).transpose(0, 2, 1, 3, 4)
        for k in range(NCORES)
    ]
    return np.concatenate(pieces, axis=3)


if __name__ == "__main__":
    x = np.random.randn(B, C, D, H, W).astype(np.float32)
    y = kernel(x)
    print(y.shape, y.dtype, y.sum())


# revision 39
# speedup vs baseline: 1.2471x; 1.1792x over previous
"""Boundary rendering module for Trainium2 (8 NeuronCores), fused single launch.

Computes, for x of shape (2, 4, 64, 256, 256) f32:
    mn/mx  = per-channel global min/max
    binary = ((x - mn) / (mx - mn)) > 0.5     [== (x - mn) > 0.5*(mx - mn)]
    dilated = 3x3x3 binary dilation of binary (SAME padding)
    out    = dilated - binary

Sharding: H (=256) split into 8 chunks of 32 rows, one per NeuronCore.
Each core receives its 32 rows plus one halo row on each side (global
edges padded with -1e30 so the halo mask is 0).  On-core layout puts
(B, D) = 128 on the SBUF partition axis; (C, H, W) live on the free axis.

Single launch per core:
  1. x loaded in (channel, half) chunks via gpsimd SWDGE DMA (spreads
     across all 16 SDMA engines at ~358 GB/s; the 2 HWDGE rings top out
     at ~75 GB/s).  DRAM shards use a flat per-partition layout
     [B, D, C, HA, W] -- strided 3-dim DMA APs are ~4x slower.
  2. DVE min/max partials per chunk as each load lands (overlapped).
  3. Partials cross-partition-reduced with gpsimd partition_all_reduce,
     then 3 pipelined 8-core AllReduce(max) collectives over [mx, -mn]
     groups (c0 | c1 | c2,c3) through DRAM bounce buffers; results
     broadcast back to 128 partitions with gpsimd partition_broadcast.
     (The first mesh collective cannot begin before a ~75us subsystem
     floor; splitting lets channel 0's thresholds land right as the
     min/max passes finish.)
  4. Mask + H-dilation on DVE (bf16), W+D dilation as banded PE matmuls
     accumulating a neighbor count in PSUM, minus 16*binary, then a
     saturated sigmoid on ScalarE emits exact {0.0, 1.0}.
  5. Stores via gpsimd SWDGE in 1 MiB chunks.
Compute runs at half-channel granularity (16 own rows) with
double-buffered mask/dilation tiles so DVE/PE/ScalarE/DMA pipeline.
"""

import os
import sys

import numpy as np

for _p in ("/opt/trn_rl_repo", "/root/.axon_site/_ro/trn_rl_repo"):
    if os.path.isdir(_p) and _p not in sys.path:
        sys.path.insert(0, _p)

import ml_dtypes

B, C, D, H, W = 2, 4, 64, 256, 256
NCORES = 8
HS = H // NCORES  # 32 own rows per core
HA = HS + 2  # rows incl halo
HPAD = np.float32(-1e30)  # halo pad at global H edges -> mask 0

MHW = 258  # mH row width: 256 data cols + 2 zero pad cols
# half-channel mH: 17 rows (1 pad + 16 data) + slack for dw=+1 AP views
MHLEN = 17 * MHW + 2

# flat per-partition sizes: DRAM shards use layout [B, D, C, H', W] so each
# partition (b, d) owns one contiguous run -> 2-dim DMA APs.  Strided 3-dim
# APs run ~4x slower through SWDGE (measured 244us vs 56us for the 17.8 MiB
# load) and defeat chunk-completion staggering.
XPP = C * HA * W  # 34816 f32 per partition (input incl halo)
OPP = C * HS * W  # 32768 f32 per partition (output)

_TWO_PHASE = False
_CACHE = {}


def _consts():
    bd = np.arange(128)
    b = bd // D
    d = bd % D
    A = (b[:, None] == b[None, :]) & (np.abs(d[:, None] - d[None, :]) <= 1)
    A = A.astype(ml_dtypes.bfloat16)
    negI = (-16.0 * np.eye(128)).astype(ml_dtypes.bfloat16)
    I128 = np.eye(128, dtype=np.float32)
    return A, negI, I128


def _build():
    import concourse.bass as bass
    import concourse.bacc as bacc
    import concourse.bass_isa as bass_isa
    import concourse.mybir as mybir
    import concourse.tile as tile
    from contextlib import ExitStack

    f32 = mybir.dt.float32
    bf16 = mybir.dt.bfloat16
    Alu = mybir.AluOpType

    nc = bacc.Bacc(
        "TRN2",
        target_bir_lowering=False,
        debug=False,
        num_devices=NCORES,
    )

    xs = nc.dram_tensor("xs", [B, D, XPP], f32, kind="ExternalInput")
    out = nc.dram_tensor("out", [B, D, OPP], f32, kind="ExternalOutput")
    A_np, negI_np, I_np = _consts()
    bandA_d = nc.inline_tensor(A_np, name="bandA")
    negI_d = nc.inline_tensor(negI_np, name="negI")
    ident_d = nc.inline_tensor(I_np, name="ident")

    # partition axis = (b, d) = 128; the (b, d) DRAM dims merge into a single
    # 128-partition dim during AP optimization since b's stride = 64 * d's.
    xsa = xs.ap()
    outa = out.ap()

    with ExitStack() as ctx:
        tc = ctx.enter_context(tile.TileContext(nc))
        pers = ctx.enter_context(tc.tile_pool(name="pers", bufs=1))
        maskp = ctx.enter_context(tc.tile_pool(name="mask", bufs=2))
        stagp = ctx.enter_context(tc.tile_pool(name="stag", bufs=2))
        psump = ctx.enter_context(tc.tile_pool(name="psum", bufs=4, space="PSUM"))
        dramp = ctx.enter_context(tc.tile_pool(name="dram", bufs=1, space="DRAM"))

        x_all = pers.tile([128, C, HA, W], f32)  # 136 KiB / partition
        pmax = pers.tile([128, 16], f32)
        pmin = pers.tile([128, 16], f32)
        red8 = pers.tile([128, 8], f32)  # per channel c: [mx_c, -mn_c] at 2c
        par8 = pers.tile([128, 8], f32)  # cross-partition reduced partials
        s1v = pers.tile([128, 8], f32)  # allreduced vals on partition 0
        gv8 = pers.tile([128, 8], f32)  # broadcast [mx, -mn] x 4 channels
        mnv = pers.tile([128, 4], f32)  # mn per channel
        h4 = pers.tile([128, 4], f32)  # 0.5*(mx-mn) per channel
        At = pers.tile([128, 128], bf16)
        Nt = pers.tile([128, 128], bf16)
        sel_bias = pers.tile([128, 2], f32)  # col 0: sigmoid bias; also dummy-AR payload

        nc.vector.memset(sel_bias[:, :], -100.0)
        nc.sync.dma_start(out=At[:, :], in_=bandA_d.ap())
        nc.sync.dma_start(out=Nt[:, :], in_=negI_d.ap())

        # ---- bulk loads: 8 chunks of 2.2 MiB on the gpsimd SWDGE queue ----
        # FIFO ring order staggers completions so per-chunk reduces overlap.
        xv = x_all[:].rearrange("p c h w -> p (c h w)")
        # channel 0 loads in 4 half-chunks so the first DVE reduce starts
        # ~4us earlier; later channels use full 17-row chunks
        for c in range(C):
            nsub = 2 if c == 0 else 1
            for hf in range(2):
                o = c * (HA * W) + hf * 17 * W
                step = (17 * W) // nsub
                for u in range(nsub):
                    nc.gpsimd.dma_start(
                        out=xv[:, o + u * step : o + (u + 1) * step],
                        in_=xsa[:, :, o + u * step : o + (u + 1) * step],
                    )
        # zero the pad columns of both mH pool buffers once, on gpsimd while
        # it idles behind the load descriptor generation (off DVE's path)
        for _ in range(2):
            mh0 = maskp.tile([128, MHLEN], bf16, tag="mh")
            nc.gpsimd.memset(mh0[:, :], 0.0)

        # ---- phase 1: min/max partials + 3 pipelined AllReduces ----
        # Channel 0 gets its own AllReduce so the first mesh collective
        # starts as early as possible (its ~27us latency hides under the
        # remaining min/max passes); later groups' meshes pipeline ~10us
        # apart behind it.
        groups = [(0,), (1,), (2, 3)]
        # own-row subranges per channel; channel 0 is split finer so the
        # first reduces start as soon as its first half-chunk lands
        subranges = {0: [(1, 8), (8, 17), (17, 25), (25, 33)]}
        for g, chans in enumerate(groups):
            for c in chans:
                subs = subranges.get(c, [(1, 17), (17, 33)])
                for k, (ra, rb) in enumerate(subs):
                    o1 = c * (HA * W) + ra * W
                    rows = xv[:, o1 : o1 + (rb - ra) * W]
                    nc.vector.tensor_reduce(
                        out=pmax[:, 4 * c + k : 4 * c + k + 1],
                        in_=rows,
                        axis=mybir.AxisListType.X,
                        op=Alu.max,
                    )
                    nc.vector.tensor_reduce(
                        out=pmin[:, 4 * c + k : 4 * c + k + 1],
                        in_=rows,
                        axis=mybir.AxisListType.X,
                        op=Alu.min,
                    )
                nc.vector.tensor_reduce(
                    out=red8[:, 2 * c : 2 * c + 1],
                    in_=pmax[:, 4 * c : 4 * c + len(subs)],
                    axis=mybir.AxisListType.X,
                    op=Alu.max,
                )
                nc.vector.tensor_reduce(
                    out=red8[:, 2 * c + 1 : 2 * c + 2],
                    in_=pmin[:, 4 * c : 4 * c + len(subs)],
                    axis=mybir.AxisListType.X,
                    op=Alu.min,
                )
                # negate the min so a single AllReduce(max) covers both
                nc.vector.tensor_scalar_mul(
                    red8[:, 2 * c + 1 : 2 * c + 2],
                    red8[:, 2 * c + 1 : 2 * c + 2],
                    -1.0,
                )
            c0 = chans[0]
            wid = 2 * len(chans)
            # cross-partition max of the group's partials on gpsimd, then
            # DMA out partition 0 and AllReduce(max) across the 8 cores.
            nc.gpsimd.partition_all_reduce(
                out_ap=par8[:, 2 * c0 : 2 * c0 + wid],
                in_ap=red8[:, 2 * c0 : 2 * c0 + wid],
                channels=128,
                reduce_op=bass_isa.ReduceOp.max,
            )
            cc_in = dramp.tile([1, wid], f32, tag=f"ccin{g}")
            cc_out = dramp.tile([1, wid], f32, tag=f"ccout{g}")
            nc.scalar.dma_start(
                out=cc_in[:, :], in_=par8[0:1, 2 * c0 : 2 * c0 + wid]
            )
            nc.gpsimd.collective_compute(
                "AllReduce",
                Alu.max,
                replica_groups=[list(range(NCORES))],
                ins=[cc_in[:, :].opt()],
                outs=[cc_out[:, :].opt()],
            )
            nc.sync.dma_start(
                out=s1v[0:1, 2 * c0 : 2 * c0 + wid],
                in_=cc_out[:, :],
            )

        # ---- phase 2: per group: thresholds, mask, dilate, boundary ----
        # all broadcasts emitted first: gpsimd's in-order program must not
        # park a later group's broadcast behind an earlier group's stores
        for g, chans in enumerate(groups):
            c0 = chans[0]
            wid = 2 * len(chans)
            nc.gpsimd.partition_broadcast(
                out_ap=gv8[:, 2 * c0 : 2 * c0 + wid],
                in_ap=s1v[0:1, 2 * c0 : 2 * c0 + wid],
            )
        for g, chans in enumerate(groups):
            for c in chans:
                nc.vector.tensor_scalar_mul(
                    mnv[:, c : c + 1],
                    gv8[:, 2 * c + 1 : 2 * c + 2],
                    -1.0,
                )
                nc.vector.tensor_add(
                    h4[:, c : c + 1],
                    gv8[:, 2 * c : 2 * c + 1],
                    gv8[:, 2 * c + 1 : 2 * c + 2],
                )
                nc.vector.tensor_scalar_mul(h4[:, c : c + 1], h4[:, c : c + 1], 0.5)
            for c in chans:
                for hf in range(2):
                    # binm rows 0..17 = x halo rows 16*hf .. 16*hf+17
                    binm = maskp.tile([128, 18, W], bf16, tag="bin")
                    mH = maskp.tile([128, MHLEN], bf16, tag="mh")
                    nc.vector.tensor_scalar(
                        out=binm[:, :, :],
                        in0=x_all[:, c, 16 * hf : 16 * hf + 18, :],
                        scalar1=mnv[:, c : c + 1],
                        scalar2=h4[:, c : c + 1],
                        op0=Alu.subtract,
                        op1=Alu.is_gt,
                    )
                    mHd = mH[:, MHW : MHW + 16 * MHW].rearrange(
                        "p (r z) -> p r z", z=MHW
                    )[:, :, 0:W]
                    # H-max in two row-ranges (0:9 covers PE groups tp=0's
                    # mH reads incl the row-9 overlap) so the first PE
                    # groups start before the second range finishes
                    for ra, rb in ((0, 9), (9, 16)):
                        nc.vector.tensor_tensor(
                            out=mHd[:, ra:rb, :],
                            in0=binm[:, ra : rb, :],
                            in1=binm[:, ra + 2 : rb + 2, :],
                            op=Alu.max,
                        )
                        nc.vector.tensor_tensor(
                            out=mHd[:, ra:rb, :],
                            in0=mHd[:, ra:rb, :],
                            in1=binm[:, ra + 1 : rb + 1, :],
                            op=Alu.max,
                        )
                    for tp in range(2):  # 8 own rows per staging buffer
                        stg = stagp.tile([128, 2048], f32, tag="st")
                        for t in range(2):  # 4 own rows per PSUM tile
                            ps = psump.tile([128, 1024], f32, tag="ps")
                            for s in range(2):  # one PSUM bank = 2 rows
                                R = 8 * tp + 4 * t + 2 * s
                                pslice = ps[:, 512 * s : 512 * s + 512]
                                for jj, dw in enumerate((-1, 0, 1)):
                                    off = (R + 1) * MHW + dw
                                    rhs = mH[:, off : off + 2 * MHW].rearrange(
                                        "p (r z) -> p r z", z=MHW
                                    )[:, :, 0:W]
                                    nc.tensor.matmul(
                                        pslice,
                                        At[:, :],
                                        rhs,
                                        start=(jj == 0),
                                        stop=False,
                                    )
                                nc.tensor.matmul(
                                    pslice,
                                    Nt[:, :],
                                    binm[:, 1 + R : 3 + R, :],
                                    start=False,
                                    stop=True,
                                )
                            nc.scalar.activation(
                                out=stg[:, 1024 * t : 1024 * t + 1024],
                                in_=ps[:, :],
                                func=mybir.ActivationFunctionType.Sigmoid,
                                bias=sel_bias[:, 0:1],
                                scale=200.0,
                            )
                        r0 = 16 * hf + 8 * tp  # own-row base in the shard
                        oo = c * (HS * W) + r0 * W
                        nc.gpsimd.dma_start(
                            out=outa[:, :, oo : oo + 8 * W],
                            in_=stg[:, :],
                        )

    nc.compile()
    return nc


def _get_nc():
    if "nc" not in _CACHE:
        _CACHE["nc"] = _build()
    return _CACHE["nc"]


def _make_in_maps(x: np.ndarray):
    # device shards use layout [B, D, C, HA, W] (flat per partition)
    xt = np.ascontiguousarray(x.transpose(0, 2, 1, 3, 4))  # [B, D, C, H, W]
    in_maps = []
    for k in range(NCORES):
        xs = np.empty((B, D, C, HA, W), np.float32)
        lo = k * HS
        xs[:, :, :, 1 : HS + 1, :] = xt[:, :, :, lo : lo + HS, :]
        if k > 0:
            xs[:, :, :, 0, :] = xt[:, :, :, lo - 1, :]
        else:
            xs[:, :, :, 0, :] = HPAD
        if k < NCORES - 1:
            xs[:, :, :, HS + 1, :] = xt[:, :, :, lo + HS, :]
        else:
            xs[:, :, :, HS + 1, :] = HPAD
        in_maps.append({"xs": xs.reshape(B, D, XPP)})
    return in_maps


def kernel(x: np.ndarray) -> np.ndarray:
    from concourse.bass_utils import run_bass_kernel_spmd

    x = np.ascontiguousarray(np.asarray(x), dtype=np.float32)
    assert x.shape == (B, C, D, H, W)

    in_maps = _make_in_maps(x)
    res = run_bass_kernel_spmd(_get_nc(), in_maps, core_ids=list(range(NCORES)))
    # shard outs are [B, D, C, HS, W]; back to [B, C, D, HS, W], concat on H
    pieces = [
        res.results[k]["out"].reshape(B, D, C, HS, W).transpose(0, 2, 1, 3, 4)
        for k in range(NCORES)
    ]
    return np.concatenate(pieces, axis=3)


if __name__ == "__main__":
    x = np.random.randn(B, C, D, H, W).astype(np.float32)
    y = kernel(x)
    print(y.shape, y.dtype, y.sum())
